# Initial kernel scaffold
#
"""Trainium2 Bass kernel for stacked-LSTM + attention + dense head.

Model (per reference):
  3x LSTM(H=512, return_sequences) with inference BatchNorm between layers,
  attention pooling over time, then Dense(128)+BN+Dense(64)+Dense(5).
  B=128, T=512, D=128, H=512, fp32.

Strategy: data-parallel over batch (16 rows/core on 8 cores). Per core:
  - input projections xz = in @ W + b computed as full-M matmul passes
    (time dim folded into M), f32r matmuls at full PE rate
  - the sequential scans keep h transposed (hT) as stationary matmul
    operand; gates are column-permuted to [i|f|o|g] so activations fuse
  - BN folded into following-layer weights on host; attention pooling uses
    a fixed 0/1 selector matmul to reduce over time rows.

Self-contained: hardcodes shapes; no reads of reference.py/spec.json.
"""

import functools
import os
import sys

import numpy as np

B, T, D, H = 128, 512, 128, 512
NC = 8
BL = B // NC          # batch rows per core
G4 = 4 * H            # gate width 2048
EPS = 1e-3
P = 128

# column permutation: keras gate order [i|f|g|o] -> kernel order [i|f|o|g]
_PERM = np.concatenate([
    np.arange(0, 512), np.arange(512, 1024),
    np.arange(1536, 2048), np.arange(1024, 1536),
])


def _bn_fold(g, b, m, v):
    sc = g / np.sqrt(v + EPS)
    sh = b - m * sc
    return sc.astype(np.float32), sh.astype(np.float32)


def prep_weights(inp):
    """Host-side constant folding. Returns dict of prepared arrays."""
    f = np.float32
    o = {}
    o['W0p'] = np.ascontiguousarray(inp['W0'][:, _PERM], f)
    o['b0p'] = np.ascontiguousarray(inp['b0'][_PERM], f)
    o['U0p'] = np.ascontiguousarray(inp['U0'][:, _PERM], f)
    o['U1p'] = np.ascontiguousarray(inp['U1'][:, _PERM], f)
    o['U2p'] = np.ascontiguousarray(inp['U2'][:, _PERM], f)
    sc0, sh0 = _bn_fold(inp['bn0_g'], inp['bn0_b'], inp['bn0_m'], inp['bn0_v'])
    o['W1p'] = np.ascontiguousarray((sc0[:, None] * inp['W1'])[:, _PERM], f)
    o['b1p'] = np.ascontiguousarray((inp['b1'] + sh0 @ inp['W1'])[_PERM], f)
    sc1, sh1 = _bn_fold(inp['bn1_g'], inp['bn1_b'], inp['bn1_m'], inp['bn1_v'])
    o['W2p'] = np.ascontiguousarray((sc1[:, None] * inp['W2'])[:, _PERM], f)
    o['b2p'] = np.ascontiguousarray((inp['b2'] + sh1 @ inp['W2'])[_PERM], f)
    o['Wa'] = np.ascontiguousarray(inp['Wa'], f)
    o['ba'] = np.ascontiguousarray(inp['ba'], f)
    # pooled = sum_t a*h2 (no 1/T); fold 1/T into Wd1
    o['Wd1p'] = np.ascontiguousarray(inp['Wd1'] / np.float32(T), f)
    o['bd1'] = np.ascontiguousarray(inp['bd1'], f)
    sc2, sh2 = _bn_fold(inp['bn2_g'], inp['bn2_b'], inp['bn2_m'], inp['bn2_v'])
    o['Wd2p'] = np.ascontiguousarray(sc2[:, None] * inp['Wd2'], f)
    o['bd2p'] = np.ascontiguousarray(inp['bd2'] + sh2 @ inp['Wd2'], f)
    o['Wd3'] = np.ascontiguousarray(inp['Wd3'], f)
    o['bd3'] = np.ascontiguousarray(inp['bd3'], f)
    # selector for summing rows (t,b) -> b : sel[p, b] = 1 if p % BL == b
    sel = np.zeros((P, BL), f)
    sel[np.arange(P), np.arange(P) % BL] = 1.0
    o['sel'] = sel
    o['ident'] = np.eye(P, dtype=f)
    return o


def _sigmoid(x):
    return 1.0 / (1.0 + np.exp(-x))


def numpy_forward(inp, t_steps=T, b_rows=B):
    """Numpy mirror of the exact kernel math (folded weights, permuted
    gates). Used to validate the host-side folds against the reference."""
    w = prep_weights(inp)
    x = np.asarray(inp['x'], np.float32)[:b_rows, :t_steps]

    def scan(xz, U):
        bsz = xz.shape[0]
        h = np.zeros((bsz, H), np.float32)
        c = np.zeros((bsz, H), np.float32)
        hs = np.empty((t_steps, bsz, H), np.float32)
        for t in range(t_steps):
            z = xz[:, t] + h @ U
            i = _sigmoid(z[:, 0:512]); f = _sigmoid(z[:, 512:1024])
            o_ = _sigmoid(z[:, 1024:1536]); g = np.tanh(z[:, 1536:2048])
            c = f * c + i * g
            h = o_ * np.tanh(c)
            hs[t] = h
        return hs  # [T, B, H]

    xz0 = np.einsum('btd,dg->btg', x, w['W0p']) + w['b0p']
    h0 = scan(xz0.transpose(1, 0, 2).transpose(1, 0, 2), w['U0p'])  # keep [B,T,*] interface
    # note: scan expects xz[:, t] -> use [B, T, G4]
    # (the double transpose above is a no-op; kept for clarity)
    h0_bt = h0.transpose(1, 0, 2)  # [B, T, H]
    xz1 = np.einsum('bth,hg->btg', h0_bt, w['W1p']) + w['b1p']
    h1 = scan(xz1, w['U1p'])
    h1_bt = h1.transpose(1, 0, 2)
    xz2 = np.einsum('bth,hg->btg', h1_bt, w['W2p']) + w['b2p']
    h2 = scan(xz2, w['U2p'])
    h2_bt = h2.transpose(1, 0, 2)  # [B, T, H]

    e = np.tanh(np.einsum('bth,hk->btk', h2_bt, w['Wa']) + w['ba'])
    s = e.sum(-1)  # [B, T]
    s = s - s.max(axis=1, keepdims=True)
    a = np.exp(s); a = a / a.sum(axis=1, keepdims=True)
    pooled = np.einsum('bt,bth->bh', a, h2_bt)  # sum, no /T (folded in Wd1p)
    d1 = np.maximum(pooled @ w['Wd1p'] + w['bd1'], 0)
    d2 = np.maximum(d1 @ w['Wd2p'] + w['bd2p'], 0)
    return d2 @ w['Wd3'] + w['bd3']


# ---------------------------------------------------------------------------
# Bass program
# ---------------------------------------------------------------------------

def build_nc(t_steps=T):
    import concourse.bacc as bacc
    import concourse.bass as bass
    import concourse.mybir as mybir
    import concourse.tile as tile
    from contextlib import ExitStack

    f32 = mybir.dt.float32
    f32r = mybir.dt.float32r
    AF = mybir.ActivationFunctionType
    OP = mybir.AluOpType
    M = t_steps * BL          # rows of the (t,b)-flattened activations
    MT = M // P               # number of 128-row tiles

    nc = bacc.Bacc("TRN2", target_bir_lowering=False, debug=False,
                   num_devices=NC)

    def din(name, shape):
        return nc.dram_tensor(name, list(shape), f32, kind="ExternalInput")

    x_d = din('x', (BL, t_steps, D))
    W0p = din('W0p', (D, G4)); b0p = din('b0p', (G4,))
    W1p = din('W1p', (H, G4)); b1p = din('b1p', (G4,))
    W2p = din('W2p', (H, G4)); b2p = din('b2p', (G4,))
    U0p = din('U0p', (H, G4)); U1p = din('U1p', (H, G4)); U2p = din('U2p', (H, G4))
    Wa = din('Wa', (H, H)); ba = din('ba', (H,))
    Wd1p = din('Wd1p', (H, P)); bd1 = din('bd1', (P,))
    Wd2p = din('Wd2p', (P, 64)); bd2p = din('bd2p', (64,))
    Wd3 = din('Wd3', (64, 5)); bd3 = din('bd3', (5,))
    sel_d = din('sel', (P, BL))
    ident_d = din('ident', (P, P))
    outT = nc.dram_tensor('outT', [5, BL], f32, kind="ExternalOutput")

    # DRAM temps
    xz_d = nc.dram_tensor('xz_buf', [M, G4], f32)
    hT_a = nc.dram_tensor('hT_a', [t_steps, P, 4 * BL], f32)
    hT_b = nc.dram_tensor('hT_b', [t_steps, P, 4 * BL], f32)
    h2rows = nc.dram_tensor('h2rows', [M, H], f32)

    NSL = [slice(n * 512, (n + 1) * 512) for n in range(4)]

    with tile.TileContext(nc) as tc:
        with ExitStack() as gctx:
            gconst = gctx.enter_context(tc.tile_pool(name="gconst", bufs=1))
            ident = gconst.tile([P, P], f32)
            nc.sync.dma_start(ident[:], ident_d[:, :])
            sel = gconst.tile([P, BL], f32)
            nc.sync.dma_start(sel[:], sel_d[:, :])

            # ---------------- xz pass ----------------
            def xz_pass(tag, kxm_ap, Kt, W_dram, b_dram):
                """xz_d[M, G4] = kxm.T @ W + b ; kxm_ap preshaped [P, Kt, M]"""
                with ExitStack() as ctx:
                    cst = ctx.enter_context(tc.tile_pool(name=f"{tag}c", bufs=1))
                    W_sb = cst.tile([P, Kt, G4], f32)
                    nc.sync.dma_start(
                        W_sb[:], W_dram.rearrange("(k p) n -> p k n", p=P))
                    brep = cst.tile([P, G4], f32)
                    nc.sync.dma_start(
                        brep[:], b_dram[None, :].to_broadcast((P, G4)))
                    io = ctx.enter_context(tc.tile_pool(name=f"{tag}io", bufs=3))
                    ps = ctx.enter_context(
                        tc.tile_pool(name=f"{tag}ps", bufs=2, space="PSUM"))
                    for m in range(MT):
                        km = io.tile([P, Kt, P], f32, tag="km")
                        nc.sync.dma_start(km[:], kxm_ap[:, :, m * P:(m + 1) * P])
                        zp = ps.tile([P, G4], f32, tag="zp")
                        for n in range(4):
                            for k in range(Kt):
                                nc.tensor.matmul(
                                    zp[:, NSL[n]],
                                    km[:, k, :].bitcast(f32r),
                                    W_sb[:, k, NSL[n]].bitcast(f32r),
                                    start=(k == 0), stop=(k == Kt - 1))
                        ob = io.tile([P, G4], f32, tag="ob")
                        nc.vector.tensor_tensor(ob[:], zp[:], brep[:], OP.add)
                        nc.sync.dma_start(xz_d[m * P:(m + 1) * P, :], ob[:])

            # ---------------- LSTM scan ----------------
            def scan(layer, U_dram, hT_out, h_rows_out=None):
                with ExitStack() as ctx:
                    cst = ctx.enter_context(tc.tile_pool(name=f"s{layer}c", bufs=1))
                    U_sb = cst.tile([P, 4, G4], f32)
                    nc.sync.dma_start(
                        U_sb[:], U_dram.rearrange("(k p) n -> p k n", p=P))
                    st = ctx.enter_context(tc.tile_pool(name=f"s{layer}s", bufs=1))
                    c_sb = st.tile([BL, H], f32)
                    nc.vector.memset(c_sb[:], 0.0)
                    hT_sb = st.tile([P, 4 * BL], f32)
                    nc.vector.memset(hT_sb[:], 0.0)
                    io = ctx.enter_context(tc.tile_pool(name=f"s{layer}io", bufs=8))
                    wk = ctx.enter_context(tc.tile_pool(name=f"s{layer}w", bufs=3))
                    psz = ctx.enter_context(
                        tc.tile_pool(name=f"s{layer}pz", bufs=1, space="PSUM"))
                    pst = ctx.enter_context(
                        tc.tile_pool(name=f"s{layer}pt", bufs=2, space="PSUM"))
                    for t in range(t_steps):
                        xz_t = io.tile([BL, G4], f32, tag="xz")
                        nc.sync.dma_start(
                            xz_t[:], xz_d[t * BL:(t + 1) * BL, :])
                        zp = psz.tile([BL, G4], f32, tag="zp")
                        for n in range(4):
                            for k in range(4):
                                nc.tensor.matmul(
                                    zp[:, NSL[n]],
                                    hT_sb[:, k * BL:(k + 1) * BL].bitcast(f32r),
                                    U_sb[:, k, NSL[n]].bitcast(f32r),
                                    start=(k == 0), stop=(k == 3))
                        z_sb = wk.tile([BL, G4], f32, tag="z")
                        nc.vector.tensor_tensor(z_sb[:], zp[:], xz_t[:], OP.add)
                        sig = wk.tile([BL, 3 * H], f32, tag="sig")
                        nc.scalar.activation(sig[:], z_sb[:, 0:3 * H], AF.Sigmoid)
                        g_sb = wk.tile([BL, H], f32, tag="g")
                        nc.scalar.activation(g_sb[:], z_sb[:, 3 * H:G4], AF.Tanh)
                        # c = f*c + i*g
                        nc.vector.tensor_tensor(
                            c_sb[:], c_sb[:], sig[:, H:2 * H], OP.mult)
                        ig = wk.tile([BL, H], f32, tag="ig")
                        nc.vector.tensor_tensor(ig[:], sig[:, 0:H], g_sb[:], OP.mult)
                        nc.vector.tensor_tensor(c_sb[:], c_sb[:], ig[:], OP.add)
                        tch = wk.tile([BL, H], f32, tag="tch")
                        nc.scalar.activation(tch[:], c_sb[:], AF.Tanh)
                        h_sb = wk.tile([BL, H], f32, tag="h")
                        nc.vector.tensor_tensor(
                            h_sb[:], sig[:, 2 * H:3 * H], tch[:], OP.mult)
                        # hT update via PE transpose
                        tp = pst.tile([P, 4 * BL], f32, tag="tp")
                        for k in range(4):
                            nc.tensor.transpose(
                                tp[:, k * BL:(k + 1) * BL],
                                h_sb[:, k * P:(k + 1) * P],
                                ident[0:BL, 0:BL])
                        nc.vector.tensor_copy(hT_sb[:], tp[:])
                        nc.sync.dma_start(hT_out[t], hT_sb[:])
                        if h_rows_out is not None:
                            nc.sync.dma_start(
                                h_rows_out[t * BL:(t + 1) * BL, :], h_sb[:])

            # ---------------- run the pipeline ----------------
            xz_pass("p0", x_d.rearrange("b t d -> d (t b)")[:, None, :], 1,
                    W0p, b0p)
            scan(0, U0p, hT_a)
            xz_pass("p1", hT_a.rearrange("t p (k b) -> p k (t b)", k=4), 4,
                    W1p, b1p)
            scan(1, U1p, hT_b)
            xz_pass("p2", hT_b.rearrange("t p (k b) -> p k (t b)", k=4), 4,
                    W2p, b2p)
            scan(2, U2p, hT_a, h2rows)

            # ---------------- attention ----------------
            with ExitStack() as ctx:
                cst = ctx.enter_context(tc.tile_pool(name="atc", bufs=1))
                Wa_sb = cst.tile([P, 4, H], f32)
                nc.sync.dma_start(
                    Wa_sb[:], Wa.rearrange("(k p) n -> p k n", p=P))
                ba_rep = cst.tile([P, H], f32)
                nc.sync.dma_start(ba_rep[:], ba[None, :].to_broadcast((P, H)))
                s_sb = cst.tile([P, MT], f32)
                io = ctx.enter_context(tc.tile_pool(name="atio", bufs=3))
                ps = ctx.enter_context(
                    tc.tile_pool(name="atps", bufs=2, space="PSUM"))
                # e-pass: s[(t,b)] = sum_k tanh(h2 @ Wa + ba)
                for m in range(MT):
                    km = io.tile([P, 8 // (T // t_steps) if False else (P // BL), 4 * BL], f32, tag="km")
                    # km: [p, t-sub(8), 4*BL] slice of hT_a rows m*8 ... not used; direct AP below
                    del km
                    kxm = io.tile([P, P // BL, 4 * BL], f32, tag="kxm")
                    nc.sync.dma_start(
                        kxm[:],
                        hT_a.rearrange("t p c -> p t c")[:, m * (P // BL):(m + 1) * (P // BL), :])
                    ep = ps.tile([P, H], f32, tag="ep")
                    for k in range(4):
                        nc.tensor.matmul(
                            ep[:],
                            kxm[:, :, k * BL:(k + 1) * BL].bitcast(f32r),
                            Wa_sb[:, k, :].bitcast(f32r),
                            start=(k == 0), stop=(k == 3))
                    e_sb = io.tile([P, H], f32, tag="e")
                    nc.vector.tensor_tensor(e_sb[:], ep[:], ba_rep[:], OP.add)
                    e_t = io.tile([P, H], f32, tag="et")
                    nc.scalar.activation(e_t[:], e_sb[:], AF.Tanh,
                                         accum_out=s_sb[:, m:m + 1])

                # transpose s (rows (t,b) layout [P, MT]) -> sT [BL, t_steps]
                sT = cst.tile([BL, t_steps], f32)
                for b in range(BL):
                    nc.sync.dma_start(
                        sT[b:b + 1, :].rearrange("o (m u) -> o u m", u=P // BL),
                        s_sb[b:P:BL, :])
                # softmax over t (free dim)
                mx = cst.tile([BL, 1], f32)
                nc.vector.reduce_max(mx[:], sT[:], axis=mybir.AxisListType.X)
                nmx = cst.tile([BL, 1], f32)
                nc.vector.tensor_scalar_mul(nmx[:], mx[:], -1.0)
                ex = cst.tile([BL, t_steps], f32)
                sm = cst.tile([BL, 1], f32)
                nc.scalar.activation(ex[:], sT[:], AF.Exp, bias=nmx[:],
                                     accum_out=sm[:])
                rs = cst.tile([BL, 1], f32)
                nc.vector.reciprocal(rs[:], sm[:])
                aT = cst.tile([BL, t_steps], f32)
                nc.vector.tensor_scalar_mul(aT[:], ex[:], rs[:])
                # scatter a back to row layout [P, MT]
                a_row = cst.tile([P, MT], f32)
                for b in range(BL):
                    nc.sync.dma_start(
                        a_row[b:P:BL, :],
                        aT[b:b + 1, :].rearrange("o (m u) -> o u m", u=P // BL))

                # pooled[b, :] = sum_rows sel * (a * h2)
                pp = ctx.enter_context(
                    tc.tile_pool(name="atpp", bufs=1, space="PSUM"))
                pooled_ps = pp.tile([BL, H], f32)
                for m in range(MT):
                    h2t = io.tile([P, H], f32, tag="h2t")
                    nc.sync.dma_start(h2t[:], h2rows[m * P:(m + 1) * P, :])
                    wrow = io.tile([P, H], f32, tag="wrow")
                    nc.vector.tensor_scalar_mul(wrow[:], h2t[:], a_row[:, m:m + 1])
                    nc.tensor.matmul(pooled_ps[:], sel[:].bitcast(f32r),
                                     wrow[:].bitcast(f32r),
                                     start=(m == 0), stop=(m == MT - 1))

                # pooledT via PE transpose
                pooled_sb = cst.tile([BL, H], f32)
                nc.vector.tensor_copy(pooled_sb[:], pooled_ps[:])
                ptp = ps.tile([P, 4 * BL], f32, tag="ptp")
                for k in range(4):
                    nc.tensor.transpose(
                        ptp[:, k * BL:(k + 1) * BL],
                        pooled_sb[:, k * P:(k + 1) * P], ident[0:BL, 0:BL])
                pooledT = cst.tile([P, 4, BL], f32)
                nc.vector.tensor_copy(
                    pooledT[:], ptp[:].rearrange("p (k b) -> p k b", k=4))

                # ---------------- dense head ----------------
                Wd1_sb = cst.tile([P, 4, P], f32)
                nc.sync.dma_start(
                    Wd1_sb[:], Wd1p.rearrange("(k p) n -> p k n", p=P))
                bd1_sb = cst.tile([P, 1], f32)
                nc.sync.dma_start(bd1_sb[:], bd1[:, None])
                Wd2_sb = cst.tile([P, 64], f32)
                nc.sync.dma_start(Wd2_sb[:], Wd2p[:, :])
                bd2_sb = cst.tile([64, 1], f32)
                nc.sync.dma_start(bd2_sb[:], bd2p[:, None])
                Wd3_sb = cst.tile([64, 5], f32)
                nc.sync.dma_start(Wd3_sb[:], Wd3[:, :])
                bd3_sb = cst.tile([5, 1], f32)
                nc.sync.dma_start(bd3_sb[:], bd3[:, None])

                d1p = ps.tile([P, BL], f32, tag="d1p")
                for k in range(4):
                    nc.tensor.matmul(d1p[:], Wd1_sb[:, k, :].bitcast(f32r),
                                     pooledT[:, k, :].bitcast(f32r),
                                     start=(k == 0), stop=(k == 3))
                d1 = cst.tile([P, BL], f32)
                nc.scalar.activation(d1[:], d1p[:], AF.Relu, bias=bd1_sb[:])
                d2p = ps.tile([64, BL], f32, tag="d2p")
                nc.tensor.matmul(d2p[:], Wd2_sb[:].bitcast(f32r),
                                 d1[:].bitcast(f32r), start=True, stop=True)
                d2 = cst.tile([64, BL], f32)
                nc.scalar.activation(d2[:], d2p[:], AF.Relu, bias=bd2_sb[:])
                d3p = ps.tile([5, BL], f32, tag="d3p")
                nc.tensor.matmul(d3p[:], Wd3_sb[:].bitcast(f32r),
                                 d2[:].bitcast(f32r), start=True, stop=True)
                d3 = cst.tile([5, BL], f32)
                nc.scalar.activation(d3[:], d3p[:], AF.Identity, bias=bd3_sb[:])
                nc.sync.dma_start(outT[:, :], d3[:])

    nc.compile()
    return nc


@functools.lru_cache(maxsize=2)
def _compiled(t_steps):
    return build_nc(t_steps)


def kernel(**inputs):
    from concourse import bass_utils
    w = prep_weights(inputs)
    x = np.ascontiguousarray(np.asarray(inputs['x'], np.float32))
    nc = _compiled(T)
    base = {k: w[k] for k in (
        'W0p', 'b0p', 'W1p', 'b1p', 'W2p', 'b2p', 'U0p', 'U1p', 'U2p',
        'Wa', 'ba', 'Wd1p', 'bd1', 'Wd2p', 'bd2p', 'Wd3', 'bd3', 'sel',
        'ident')}
    in_maps = []
    for c in range(NC):
        m = dict(base)
        m['x'] = np.ascontiguousarray(x[c * BL:(c + 1) * BL])
        in_maps.append(m)
    res = bass_utils.run_bass_kernel_spmd(nc, in_maps, core_ids=list(range(NC)))
    out = np.concatenate([np.asarray(res.results[c]['outT']).T
                          for c in range(NC)], axis=0)
    return np.ascontiguousarray(out, np.float32)


# revision 18
# speedup vs baseline: 1.1368x; 1.1368x over previous
"""Trainium2 Bass kernel for stacked-LSTM + attention + dense head.

Model (per reference):
  3x LSTM(H=512, return_sequences) with inference BatchNorm between layers,
  attention pooling over time, then Dense(128)+BN+Dense(64)+Dense(5).
  B=128, T=512, D=128, H=512, fp32.

Strategy: data-parallel over batch (16 rows/core on 8 cores). Per core:
  - input projections xz = in @ W + b computed as full-M matmul passes
    (time dim folded into M), f32r matmuls at full PE rate
  - the sequential scans keep h transposed (hT) as stationary matmul
    operand; gates are column-permuted to [i|f|o|g] so activations fuse
  - BN folded into following-layer weights on host; attention pooling uses
    a fixed 0/1 selector matmul to reduce over time rows.

Self-contained: hardcodes shapes; no reads of reference.py/spec.json.
"""

import functools
import os
import sys

import numpy as np

B, T, D, H = 128, 512, 128, 512
NC = 8
BL = B // NC          # batch rows per core
G4 = 4 * H            # gate width 2048
EPS = 1e-3
P = 128

# column permutation: keras gate order [i|f|g|o] -> kernel order [i|f|o|g]
_PERM = np.concatenate([
    np.arange(0, 512), np.arange(512, 1024),
    np.arange(1536, 2048), np.arange(1024, 1536),
])


def _bn_fold(g, b, m, v):
    sc = g / np.sqrt(v + EPS)
    sh = b - m * sc
    return sc.astype(np.float32), sh.astype(np.float32)


def prep_weights(inp):
    """Host-side constant folding. Returns dict of prepared arrays."""
    f = np.float32
    o = {}
    o['W0p'] = np.ascontiguousarray(inp['W0'][:, _PERM], f)
    o['b0p'] = np.ascontiguousarray(inp['b0'][_PERM], f)
    o['U0p'] = np.ascontiguousarray(inp['U0'][:, _PERM], f)
    o['U1p'] = np.ascontiguousarray(inp['U1'][:, _PERM], f)
    o['U2p'] = np.ascontiguousarray(inp['U2'][:, _PERM], f)
    sc0, sh0 = _bn_fold(inp['bn0_g'], inp['bn0_b'], inp['bn0_m'], inp['bn0_v'])
    o['W1p'] = np.ascontiguousarray((sc0[:, None] * inp['W1'])[:, _PERM], f)
    o['b1p'] = np.ascontiguousarray((inp['b1'] + sh0 @ inp['W1'])[_PERM], f)
    sc1, sh1 = _bn_fold(inp['bn1_g'], inp['bn1_b'], inp['bn1_m'], inp['bn1_v'])
    o['W2p'] = np.ascontiguousarray((sc1[:, None] * inp['W2'])[:, _PERM], f)
    o['b2p'] = np.ascontiguousarray((inp['b2'] + sh1 @ inp['W2'])[_PERM], f)
    o['Wa'] = np.ascontiguousarray(inp['Wa'], f)
    o['ba'] = np.ascontiguousarray(inp['ba'], f)
    # pooled = sum_t a*h2 (no 1/T); fold 1/T into Wd1
    o['Wd1p'] = np.ascontiguousarray(inp['Wd1'] / np.float32(T), f)
    o['bd1'] = np.ascontiguousarray(inp['bd1'], f)
    sc2, sh2 = _bn_fold(inp['bn2_g'], inp['bn2_b'], inp['bn2_m'], inp['bn2_v'])
    o['Wd2p'] = np.ascontiguousarray(sc2[:, None] * inp['Wd2'], f)
    o['bd2p'] = np.ascontiguousarray(inp['bd2'] + sh2 @ inp['Wd2'], f)
    o['Wd3'] = np.ascontiguousarray(inp['Wd3'], f)
    o['bd3'] = np.ascontiguousarray(inp['bd3'], f)
    # selector for summing rows (t,b) -> b : sel[p, b] = 1 if p % BL == b
    sel = np.zeros((P, BL), f)
    sel[np.arange(P), np.arange(P) % BL] = 1.0
    o['sel'] = sel
    o['ident'] = np.eye(P, dtype=f)
    return o


def _sigmoid(x):
    return 1.0 / (1.0 + np.exp(-x))


def numpy_forward(inp, t_steps=T, b_rows=B):
    """Numpy mirror of the exact kernel math (folded weights, permuted
    gates). Used to validate the host-side folds against the reference."""
    w = prep_weights(inp)
    x = np.asarray(inp['x'], np.float32)[:b_rows, :t_steps]

    def scan(xz, U):
        bsz = xz.shape[0]
        h = np.zeros((bsz, H), np.float32)
        c = np.zeros((bsz, H), np.float32)
        hs = np.empty((t_steps, bsz, H), np.float32)
        for t in range(t_steps):
            z = xz[:, t] + h @ U
            i = _sigmoid(z[:, 0:512]); f = _sigmoid(z[:, 512:1024])
            o_ = _sigmoid(z[:, 1024:1536]); g = np.tanh(z[:, 1536:2048])
            c = f * c + i * g
            h = o_ * np.tanh(c)
            hs[t] = h
        return hs  # [T, B, H]

    xz0 = np.einsum('btd,dg->btg', x, w['W0p']) + w['b0p']
    h0 = scan(xz0.transpose(1, 0, 2).transpose(1, 0, 2), w['U0p'])  # keep [B,T,*] interface
    # note: scan expects xz[:, t] -> use [B, T, G4]
    # (the double transpose above is a no-op; kept for clarity)
    h0_bt = h0.transpose(1, 0, 2)  # [B, T, H]
    xz1 = np.einsum('bth,hg->btg', h0_bt, w['W1p']) + w['b1p']
    h1 = scan(xz1, w['U1p'])
    h1_bt = h1.transpose(1, 0, 2)
    xz2 = np.einsum('bth,hg->btg', h1_bt, w['W2p']) + w['b2p']
    h2 = scan(xz2, w['U2p'])
    h2_bt = h2.transpose(1, 0, 2)  # [B, T, H]

    e = np.tanh(np.einsum('bth,hk->btk', h2_bt, w['Wa']) + w['ba'])
    s = e.sum(-1)  # [B, T]
    s = s - s.max(axis=1, keepdims=True)
    a = np.exp(s); a = a / a.sum(axis=1, keepdims=True)
    pooled = np.einsum('bt,bth->bh', a, h2_bt)  # sum, no /T (folded in Wd1p)
    d1 = np.maximum(pooled @ w['Wd1p'] + w['bd1'], 0)
    d2 = np.maximum(d1 @ w['Wd2p'] + w['bd2p'], 0)
    return d2 @ w['Wd3'] + w['bd3']


# ---------------------------------------------------------------------------
# Bass program
# ---------------------------------------------------------------------------

def build_nc(t_steps=T):
    import concourse.bacc as bacc
    import concourse.bass as bass
    import concourse.mybir as mybir
    import concourse.tile as tile
    from contextlib import ExitStack

    f32 = mybir.dt.float32
    f32r = mybir.dt.float32r
    AF = mybir.ActivationFunctionType
    OP = mybir.AluOpType
    M = t_steps * BL          # rows of the (t,b)-flattened activations
    MT = M // P               # number of 128-row tiles

    nc = bacc.Bacc("TRN2", target_bir_lowering=False, debug=False,
                   num_devices=NC)

    def din(name, shape):
        return nc.dram_tensor(name, list(shape), f32, kind="ExternalInput")

    x_d = din('xT', (D, t_steps, BL))
    W0p = din('W0p', (D, G4)); b0p = din('b0p', (G4,))
    W1p = din('W1p', (H, G4)); b1p = din('b1p', (G4,))
    W2p = din('W2p', (H, G4)); b2p = din('b2p', (G4,))
    U0p = din('U0p', (H, G4)); U1p = din('U1p', (H, G4)); U2p = din('U2p', (H, G4))
    Wa = din('Wa', (H, H)); ba = din('ba', (H,))
    Wd1p = din('Wd1p', (H, P)); bd1 = din('bd1', (P,))
    Wd2p = din('Wd2p', (P, 64)); bd2p = din('bd2p', (64,))
    Wd3 = din('Wd3', (64, 5)); bd3 = din('bd3', (5,))
    sel_d = din('sel', (P, BL))
    ident_d = din('ident', (P, P))
    outT = nc.dram_tensor('outT', [5, BL], f32, kind="ExternalOutput")

    # DRAM temps
    xz_d = nc.dram_tensor('xz_buf', [M, G4], f32)
    hT_a = nc.dram_tensor('hT_a', [4, P, t_steps, BL], f32r)
    hT_b = nc.dram_tensor('hT_b', [4, P, t_steps, BL], f32r)
    h2rows = nc.dram_tensor('h2rows', [M, H], f32)
    s_dram = nc.dram_tensor('s_dram', [M], f32)
    a_dram = nc.dram_tensor('a_dram', [M], f32)

    NSL = [slice(n * 512, (n + 1) * 512) for n in range(4)]

    with tile.TileContext(nc) as tc:
        with ExitStack() as gctx:
            gconst = gctx.enter_context(tc.tile_pool(name="gconst", bufs=1))
            ident = gconst.tile([P, P], f32)
            nc.sync.dma_start(ident[:], ident_d[:, :])
            sel = gconst.tile([P, BL], f32)
            nc.sync.dma_start(sel[:], sel_d[:, :])
            sel_r = gconst.tile([P, BL], f32r)
            nc.any.tensor_copy(sel_r[:], sel[:])

            def load_r(pool, dram_ap, shape, name):
                # DMA to f32 staging then round-copy into an f32r tile
                stg = pool.tile(shape, f32, name=name + "_stg")
                nc.sync.dma_start(stg[:], dram_ap)
                t = pool.tile(shape, f32r, name=name)
                nc.any.tensor_copy(t[:], stg[:])
                return t

            # ---------------- xz pass ----------------
            TPB = P // BL  # timesteps per 128-row tile

            def xz_pass(tag, kxm_ap, Kt, W_dram, b_dram):
                """xz_d[M, G4] = kxm.T @ W + b ; kxm_ap preshaped
                [P, Kt, t_steps, BL] (partition, ktile, t, b)"""
                with ExitStack() as ctx:
                    cst = ctx.enter_context(tc.tile_pool(name=f"{tag}c", bufs=1))
                    W_sb = load_r(cst, W_dram.rearrange("(k p) n -> p k n", p=P),
                                  [P, Kt, G4], f"{tag}W")
                    brep = cst.tile([P, G4], f32)
                    nc.sync.dma_start(
                        brep[:], b_dram[None, :].to_broadcast((P, G4)))
                    io = ctx.enter_context(tc.tile_pool(name=f"{tag}io", bufs=3))
                    ps = ctx.enter_context(
                        tc.tile_pool(name=f"{tag}ps", bufs=2, space="PSUM"))
                    for m in range(MT):
                        km = io.tile([P, Kt, P], f32r, tag="km")
                        if Kt == 1:
                            km_s = io.tile([P, Kt, P], f32, tag="km_s")
                            nc.sync.dma_start(
                                km_s[:, 0, :].rearrange("p (t b) -> p t b", b=BL),
                                kxm_ap[:, 0, m * TPB:(m + 1) * TPB, :])
                            nc.any.tensor_copy(km[:], km_s[:])
                        else:
                            nc.sync.dma_start(
                                km[:].rearrange("p k (t b) -> p k t b", b=BL),
                                kxm_ap[:, :, m * TPB:(m + 1) * TPB, :])
                        zp = ps.tile([P, G4], f32, tag="zp")
                        for n in range(4):
                            for k in range(Kt):
                                nc.tensor.matmul(
                                    zp[:, NSL[n]],
                                    km[:, k, :], W_sb[:, k, NSL[n]],
                                    start=(k == 0), stop=(k == Kt - 1))
                        ob = io.tile([P, G4], f32, tag="ob")
                        nc.vector.tensor_tensor(ob[:], zp[:], brep[:], OP.add)
                        nc.sync.dma_start(xz_d[m * P:(m + 1) * P, :], ob[:])

            # ---------------- LSTM scan ----------------
            def scan(layer, U_dram, hT_out, h_rows_out=None):
                with ExitStack() as ctx:
                    cst = ctx.enter_context(tc.tile_pool(name=f"s{layer}c", bufs=1))
                    U_sb = load_r(cst, U_dram.rearrange("(k p) n -> p k n", p=P),
                                  [P, 4, G4], f"s{layer}U")
                    st = ctx.enter_context(tc.tile_pool(name=f"s{layer}s", bufs=1))
                    c_sb = st.tile([BL, H], f32)
                    nc.vector.memset(c_sb[:], 0.0)
                    hT_sb = st.tile([P, 4 * BL], f32r)
                    hT_z = st.tile([P, 4 * BL], f32, name=f"s{layer}hz")
                    nc.vector.memset(hT_z[:], 0.0)
                    nc.any.tensor_copy(hT_sb[:], hT_z[:])
                    io = ctx.enter_context(tc.tile_pool(name=f"s{layer}io", bufs=8))
                    wk = ctx.enter_context(tc.tile_pool(name=f"s{layer}w", bufs=3))
                    psz = ctx.enter_context(
                        tc.tile_pool(name=f"s{layer}pz", bufs=1, space="PSUM"))
                    pst = ctx.enter_context(
                        tc.tile_pool(name=f"s{layer}pt", bufs=2, space="PSUM"))
                    for t in range(t_steps):
                        xz_t = io.tile([BL, G4], f32, tag="xz")
                        nc.sync.dma_start(
                            xz_t[:], xz_d[t * BL:(t + 1) * BL, :])
                        zp = psz.tile([BL, G4], f32, tag="zp")
                        for n in range(4):
                            for k in range(4):
                                nc.tensor.matmul(
                                    zp[:, NSL[n]],
                                    hT_sb[:, k * BL:(k + 1) * BL],
                                    U_sb[:, k, NSL[n]],
                                    start=(k == 0), stop=(k == 3))
                        z_sb = wk.tile([BL, G4], f32, tag="z")
                        nc.vector.tensor_tensor(z_sb[:], zp[:], xz_t[:], OP.add)
                        sig = wk.tile([BL, 3 * H], f32, tag="sig")
                        nc.scalar.activation(sig[:], z_sb[:, 0:3 * H], AF.Sigmoid)
                        g_sb = wk.tile([BL, H], f32, tag="g")
                        nc.scalar.activation(g_sb[:], z_sb[:, 3 * H:G4], AF.Tanh)
                        # c = f*c + i*g
                        nc.vector.tensor_tensor(
                            c_sb[:], c_sb[:], sig[:, H:2 * H], OP.mult)
                        ig = wk.tile([BL, H], f32, tag="ig")
                        nc.vector.tensor_tensor(ig[:], sig[:, 0:H], g_sb[:], OP.mult)
                        nc.vector.tensor_tensor(c_sb[:], c_sb[:], ig[:], OP.add)
                        tch = wk.tile([BL, H], f32, tag="tch")
                        nc.scalar.activation(tch[:], c_sb[:], AF.Tanh)
                        h_sb = wk.tile([BL, H], f32, tag="h")
                        nc.vector.tensor_tensor(
                            h_sb[:], sig[:, 2 * H:3 * H], tch[:], OP.mult)
                        # hT update via PE transpose
                        tp = pst.tile([P, 4 * BL], f32, tag="tp")
                        for k in range(4):
                            nc.tensor.transpose(
                                tp[:, k * BL:(k + 1) * BL],
                                h_sb[:, k * P:(k + 1) * P],
                                ident[0:BL, 0:BL])
                        nc.vector.tensor_copy(hT_sb[:], tp[:])
                        nc.sync.dma_start(
                            hT_out.rearrange("k p t b -> p k t b")[:, :, t, :],
                            hT_sb[:].rearrange("p (k b) -> p k b", b=BL))
                        if h_rows_out is not None:
                            nc.sync.dma_start(
                                h_rows_out[t * BL:(t + 1) * BL, :], h_sb[:])

            # ---------------- run the pipeline ----------------
            xz_pass("p0", x_d[:, None, :, :], 1, W0p, b0p)
            scan(0, U0p, hT_a)
            xz_pass("p1", hT_a.rearrange("k p t b -> p k t b"), 4,
                    W1p, b1p)
            scan(1, U1p, hT_b)
            xz_pass("p2", hT_b.rearrange("k p t b -> p k t b"), 4,
                    W2p, b2p)
            scan(2, U2p, hT_a, h2rows)

            # ---------------- attention ----------------
            with ExitStack() as ctx:
                cst = ctx.enter_context(tc.tile_pool(name="atc", bufs=1))
                Wa_sb = load_r(cst, Wa.rearrange("(k p) n -> p k n", p=P),
                               [P, 4, H], "atWa")
                ba_rep = cst.tile([P, H], f32)
                nc.sync.dma_start(ba_rep[:], ba[None, :].to_broadcast((P, H)))
                s_sb = cst.tile([P, MT], f32)
                io = ctx.enter_context(tc.tile_pool(name="atio", bufs=3))
                ps = ctx.enter_context(
                    tc.tile_pool(name="atps", bufs=2, space="PSUM"))
                # e-pass: s[(t,b)] = sum_k tanh(h2 @ Wa + ba)
                for m in range(MT):
                    kxm = io.tile([P, 4, TPB, BL], f32r, tag="kxm")
                    for k in range(4):
                        nc.sync.dma_start(
                            kxm[:, k],
                            hT_a[k, :, m * TPB:(m + 1) * TPB, :])
                    ep = ps.tile([P, H], f32, tag="ep")
                    for k in range(4):
                        nc.tensor.matmul(
                            ep[:],
                            kxm[:, k], Wa_sb[:, k, :],
                            start=(k == 0), stop=(k == 3))
                    e_sb = io.tile([P, H], f32, tag="e")
                    nc.vector.tensor_tensor(e_sb[:], ep[:], ba_rep[:], OP.add)
                    e_t = io.tile([P, H], f32, tag="et")
                    nc.scalar.activation(e_t[:], e_sb[:], AF.Tanh,
                                         accum_out=s_sb[:, m:m + 1])

                # transpose s (rows (t,b) layout [P, MT]) -> sT [BL, t_steps]
                # via a flat DRAM buffer: row index r = t*BL + b, so the flat
                # buffer reinterprets as [t, b] for free
                nc.sync.dma_start(
                    s_dram.rearrange("(m p) -> p m", p=P), s_sb[:])
                sT = cst.tile([BL, t_steps], f32)
                nc.sync.dma_start(
                    sT[:], s_dram.rearrange("(t b) -> b t", b=BL))
                # softmax over t (free dim)
                mx = cst.tile([BL, 1], f32)
                nc.vector.reduce_max(mx[:], sT[:], axis=mybir.AxisListType.X)
                nmx = cst.tile([BL, 1], f32)
                nc.vector.tensor_scalar_mul(nmx[:], mx[:], -1.0)
                ex = cst.tile([BL, t_steps], f32)
                sm = cst.tile([BL, 1], f32)
                nc.scalar.activation(ex[:], sT[:], AF.Exp, bias=nmx[:],
                                     accum_out=sm[:])
                rs = cst.tile([BL, 1], f32)
                nc.vector.reciprocal(rs[:], sm[:])
                aT = cst.tile([BL, t_steps], f32)
                nc.vector.tensor_scalar_mul(aT[:], ex[:], rs[:])
                # scatter a back to row layout [P, MT] via flat DRAM
                nc.sync.dma_start(
                    a_dram.rearrange("(t b) -> b t", b=BL), aT[:])
                a_row = cst.tile([P, MT], f32)
                nc.sync.dma_start(
                    a_row[:], a_dram.rearrange("(m p) -> p m", p=P))

                # pooled[b, :] = sum_rows sel * (a * h2)
                pp = ctx.enter_context(
                    tc.tile_pool(name="atpp", bufs=1, space="PSUM"))
                ps1 = ctx.enter_context(
                    tc.tile_pool(name="atp1", bufs=1, space="PSUM"))
                pooled_ps = pp.tile([BL, H], f32)
                for m in range(MT):
                    h2t = io.tile([P, H], f32, tag="h2t")
                    nc.sync.dma_start(h2t[:], h2rows[m * P:(m + 1) * P, :])
                    wrow = io.tile([P, H], f32r, tag="wrow")
                    nc.vector.tensor_scalar_mul(wrow[:], h2t[:], a_row[:, m:m + 1])
                    nc.tensor.matmul(pooled_ps[:], sel_r[:], wrow[:],
                                     start=(m == 0), stop=(m == MT - 1))

                # pooledT via PE transpose
                pooled_sb = cst.tile([BL, H], f32)
                nc.vector.tensor_copy(pooled_sb[:], pooled_ps[:])
                ptp = ps1.tile([P, 4 * BL], f32, tag="ptp")
                for k in range(4):
                    nc.tensor.transpose(
                        ptp[:, k * BL:(k + 1) * BL],
                        pooled_sb[:, k * P:(k + 1) * P], ident[0:BL, 0:BL])
                pooledT = cst.tile([P, 4, BL], f32r)
                nc.vector.tensor_copy(
                    pooledT[:], ptp[:].rearrange("p (k b) -> p k b", k=4))

                # ---------------- dense head ----------------
                Wd1_sb = load_r(cst, Wd1p.rearrange("(k p) n -> p k n", p=P),
                                [P, 4, P], "hWd1")
                bd1_sb = cst.tile([P, 1], f32)
                nc.sync.dma_start(bd1_sb[:], bd1[:, None])
                Wd2_sb = load_r(cst, Wd2p[:, :], [P, 64], "hWd2")
                bd2_sb = cst.tile([64, 1], f32)
                nc.sync.dma_start(bd2_sb[:], bd2p[:, None])
                Wd3_sb = load_r(cst, Wd3[:, :], [64, 5], "hWd3")
                bd3_sb = cst.tile([5, 1], f32)
                nc.sync.dma_start(bd3_sb[:], bd3[:, None])

                d1p = ps1.tile([P, BL], f32, tag="d1p")
                for k in range(4):
                    nc.tensor.matmul(d1p[:], Wd1_sb[:, k, :],
                                     pooledT[:, k, :],
                                     start=(k == 0), stop=(k == 3))
                d1 = cst.tile([P, BL], f32r)
                nc.scalar.activation(d1[:], d1p[:], AF.Relu, bias=bd1_sb[:])
                d2p = ps1.tile([64, BL], f32, tag="d2p")
                nc.tensor.matmul(d2p[:], Wd2_sb[:], d1[:],
                                 start=True, stop=True)
                d2 = cst.tile([64, BL], f32r)
                nc.scalar.activation(d2[:], d2p[:], AF.Relu, bias=bd2_sb[:])
                d3p = ps1.tile([5, BL], f32, tag="d3p")
                nc.tensor.matmul(d3p[:], Wd3_sb[:], d2[:],
                                 start=True, stop=True)
                d3 = cst.tile([5, BL], f32)
                nc.scalar.activation(d3[:], d3p[:], AF.Identity, bias=bd3_sb[:])
                nc.sync.dma_start(outT[:, :], d3[:])

    nc.compile()
    return nc


@functools.lru_cache(maxsize=2)
def _compiled(t_steps):
    return build_nc(t_steps)


def kernel(**inputs):
    from concourse import bass_utils
    nc = _compiled(T)
    in_maps = _make_in_maps(inputs)
    res = bass_utils.run_bass_kernel_spmd(nc, in_maps, core_ids=list(range(NC)))
    out = np.concatenate([np.asarray(res.results[c]['outT']).T
                          for c in range(NC)], axis=0)
    return np.ascontiguousarray(out, np.float32)


def _make_in_maps(inputs):
    w = prep_weights(inputs)
    x = np.ascontiguousarray(np.asarray(inputs['x'], np.float32))
    base = {k: w[k] for k in (
        'W0p', 'b0p', 'W1p', 'b1p', 'W2p', 'b2p', 'U0p', 'U1p', 'U2p',
        'Wa', 'ba', 'Wd1p', 'bd1', 'Wd2p', 'bd2p', 'Wd3', 'bd3', 'sel',
        'ident')}
    in_maps = []
    for c in range(NC):
        m = dict(base)
        m['xT'] = np.ascontiguousarray(
            x[c * BL:(c + 1) * BL].transpose(2, 1, 0))
        in_maps.append(m)
    return in_maps


def timed_run(tmpdir=None, **inputs):
    """Run with NTFF profiling; returns BassKernelResults (exec_time_ns,
    trace path)."""
    from concourse import bass_utils
    nc = _compiled(T)
    in_maps = _make_in_maps(inputs)
    res = bass_utils.run_bass_kernel_spmd(
        nc, in_maps, core_ids=list(range(NC)), trace=True, tmpdir=tmpdir)
    return res
